# revision 6
# baseline (speedup 1.0000x reference)
"""ChartCover (vq_codebook) Trainium2 kernel.

Data-parallel over batch B across 8 NeuronCores; centers/stats replicated.

Numerics: the TRN2 PE's fast fp32 path (float32r) is TF32-like (11-bit
mantissa, RNE). To keep fp32-class accuracy at ~1 cycle/row we use a
split-precision scheme:
  2*z_w.C^T  =  zh@Rhi + zh@Rlo + zl'@Rhi + (b-fold correction)
where Rhi = rne11(2C^T), Rlo = rne11(2C^T - Rhi) are host-fed, zh =
rne11(z_w) is produced by the ACT whiten (PSUM->SBUF copy, per-partition
scale/bias), and zl' = (zT*s) - zh = (z_w - zh) - b is one DVE
scalar_tensor_tensor per chunk; the constant -b@Rhi term is folded into
the cn subtraction on the host side.

Per-core pipeline (tiles of 128 rows):
  - PE transposes raw z tiles (fp32 transpose mode)
  - ACT whitens+rounds during the PSUM->SBUF copy -> zh ; DVE STT -> zl'
  - PE: 12 f32r matmuls (N=256) accumulate p = 2*z_w.C^T - b@Rhi
  - DVE: pSB = p - (cn - corr) ; max8/max_index -> argmin; ACT:
    dists = sqrt(zn - pSB) with host-fed zn bias; GPSIMD: onehot
    (iota==idx, bf16) and masks (dists<=R, u8)
  - PE: segment sums += onehot^T @ z_bf16 accumulated in PSUM (bf16 ok:
    enters new_centers scaled by TAU=0.01)
Host: shard/gather, zn = ||z_w||^2 rows, bincount(idx) for counts, affine
whitening fixup of raw sums, EMA center update (M x D, tiny).
"""

from contextlib import ExitStack

import numpy as np

import concourse.bacc as bacc
import concourse.tile as tile
from concourse import mybir
from concourse.bass_utils import run_bass_kernel_spmd

B, D, M = 131072, 512, 256
R = 32.0
TAU = 0.01
EPS = 1e-6
NCORES = 8
P = 128
NCH = D // P  # 4 contraction chunks
F32 = mybir.dt.float32
F32R = mybir.dt.float32r
BF16 = mybir.dt.bfloat16
AF = mybir.ActivationFunctionType
OP = mybir.AluOpType


def build_program(BC):
    NT = BC // P
    assert NT <= P
    nc = bacc.Bacc("TRN2", target_bir_lowering=False, debug=False)

    z = nc.dram_tensor("z", [BC, D], F32, kind="ExternalInput").ap()
    zbf = nc.dram_tensor("zbf", [BC, D], BF16, kind="ExternalInput").ap()
    zn_in = nc.dram_tensor("zn", [NT, P], F32, kind="ExternalInput").ap()
    rhs_hi = nc.dram_tensor("rhs_hi", [NCH, P, M], F32R, kind="ExternalInput").ap()
    rhs_lo = nc.dram_tensor("rhs_lo", [NCH, P, M], F32R, kind="ExternalInput").ap()
    cnadj = nc.dram_tensor("cnadj", [P, M], F32, kind="ExternalInput").ap()
    wscale = nc.dram_tensor("wscale", [P, NCH], F32, kind="ExternalInput").ap()
    wbias = nc.dram_tensor("wbias", [P, NCH], F32, kind="ExternalInput").ap()
    iota_rep = nc.dram_tensor("iota_rep", [P, M], F32, kind="ExternalInput").ap()
    ident = nc.dram_tensor("ident", [P, P], F32, kind="ExternalInput").ap()

    dists_o = nc.dram_tensor("dists", [BC, M], F32, kind="ExternalOutput").ap()
    masks_o = nc.dram_tensor("masks", [BC, M], mybir.dt.uint8, kind="ExternalOutput").ap()
    idx_o = nc.dram_tensor("hard_idx", [BC], mybir.dt.int32, kind="ExternalOutput").ap()
    sums_o = nc.dram_tensor("sums", [M, D], F32, kind="ExternalOutput").ap()

    with tile.TileContext(nc) as tc, ExitStack() as ctx:
        const = ctx.enter_context(tc.tile_pool(name="const", bufs=1))
        znat_pool = ctx.enter_context(tc.tile_pool(name="znat", bufs=3))
        zbf_pool = ctx.enter_context(tc.tile_pool(name="zbfp", bufs=3))
        zw_pool = ctx.enter_context(tc.tile_pool(name="zw", bufs=3))
        ep_pool = ctx.enter_context(tc.tile_pool(name="ep", bufs=3))
        oh_pool = ctx.enter_context(tc.tile_pool(name="oh", bufs=3))
        small_pool = ctx.enter_context(tc.tile_pool(name="small", bufs=4))
        tr_pool = ctx.enter_context(tc.tile_pool(name="tr", bufs=2, space="PSUM"))
        pd_pool = ctx.enter_context(tc.tile_pool(name="pd", bufs=3, space="PSUM"))
        acc_pool = ctx.enter_context(tc.tile_pool(name="acc", bufs=1, space="PSUM"))

        sb_rhs_hi = const.tile([P, NCH * M], F32R, tag="rhshi")
        sb_rhs_lo = const.tile([P, NCH * M], F32R, tag="rhslo")
        for c in range(NCH):
            nc.sync.dma_start(sb_rhs_hi[:, c * M:(c + 1) * M], rhs_hi[c])
            nc.sync.dma_start(sb_rhs_lo[:, c * M:(c + 1) * M], rhs_lo[c])
        sb_cn = const.tile([P, M], F32, tag="cn")
        nc.sync.dma_start(sb_cn[:], cnadj[:])
        sb_ws = const.tile([P, NCH], F32, tag="ws")
        nc.sync.dma_start(sb_ws[:], wscale[:])
        sb_wb = const.tile([P, NCH], F32, tag="wb")
        nc.sync.dma_start(sb_wb[:], wbias[:])
        sb_iota = const.tile([P, M], F32, tag="iota")
        nc.sync.dma_start(sb_iota[:], iota_rep[:])
        sb_id = const.tile([P, P], F32, tag="id")
        nc.sync.dma_start(sb_id[:], ident[:])
        sb_zn = const.tile([P, NT], F32, tag="znc")
        nc.sync.dma_start(sb_zn[:], zn_in.rearrange("t b -> b t"))
        idx_stage = const.tile([P, P], F32, tag="idxstage")

        sums_ps = [acc_pool.tile([P, D], F32, tag=f"sums{i}", name=f"sums_ps{i}")
                   for i in range(2)]

        for t in range(NT):
            zt = znat_pool.tile([P, D], F32, tag="znat", name="zt")
            nc.sync.dma_start(zt[:], z[t * P:(t + 1) * P, :])
            zb = zbf_pool.tile([P, D], BF16, tag="zbf", name="zb")
            nc.sync.dma_start(zb[:], zbf[t * P:(t + 1) * P, :])

            ps_tr = tr_pool.tile([P, D], F32, tag="tr", name="ps_tr")
            for c in range(NCH):
                nc.tensor.transpose(
                    ps_tr[:, c * P:(c + 1) * P], zt[:, c * P:(c + 1) * P], sb_id[:]
                )
            zh = zw_pool.tile([P, D], F32R, tag="zh", name="zh")
            zl = zw_pool.tile([P, D], F32R, tag="zl", name="zl")
            for c in range(NCH):
                sl = slice(c * P, (c + 1) * P)
                nc.scalar.activation(
                    zh[:, sl], ps_tr[:, sl], AF.Identity,
                    bias=sb_wb[:, c:c + 1], scale=sb_ws[:, c:c + 1],
                )
                nc.vector.scalar_tensor_tensor(
                    out=zl[:, sl], in0=ps_tr[:, sl], scalar=sb_ws[:, c:c + 1],
                    in1=zh[:, sl], op0=OP.mult, op1=OP.subtract,
                )
            pd = pd_pool.tile([P, M], F32, tag="pd", name="pd")
            for c in range(NCH):
                sl = slice(c * P, (c + 1) * P)
                ms = slice(c * M, (c + 1) * M)
                nc.tensor.matmul(
                    pd[:], zh[:, sl], sb_rhs_hi[:, ms],
                    start=(c == 0), stop=False, skip_group_check=True,
                )
                nc.tensor.matmul(
                    pd[:], zh[:, sl], sb_rhs_lo[:, ms],
                    start=False, stop=False, skip_group_check=True,
                )
                nc.tensor.matmul(
                    pd[:], zl[:, sl], sb_rhs_hi[:, ms],
                    start=False, stop=(c == NCH - 1), skip_group_check=True,
                )
            # epilogue
            psb = ep_pool.tile([P, M], F32, tag="psb", name="psb")
            nc.vector.tensor_tensor(
                out=psb[:], in0=pd[:], in1=sb_cn[:], op=OP.subtract
            )
            maxv = small_pool.tile([P, 8], F32, tag="maxv", name="maxv")
            nc.vector.max(maxv[:], psb[:])
            idx8 = small_pool.tile([P, 8], mybir.dt.uint32, tag="idx8", name="idx8")
            nc.vector.max_index(idx8[:], maxv[:], psb[:])
            nc.vector.tensor_copy(idx_stage[:, t:t + 1], idx8[:, 0:1])
            dsb = ep_pool.tile([P, M], F32, tag="dist", name="dsb")
            nc.scalar.activation(
                dsb[:], psb[:], AF.Sqrt, bias=sb_zn[:, t:t + 1], scale=-1.0
            )
            oh = oh_pool.tile([P, M], BF16, tag="oh", name="oh")
            nc.gpsimd.tensor_scalar(
                out=oh[:], in0=sb_iota[:], scalar1=idx_stage[:, t:t + 1],
                scalar2=None, op0=OP.is_equal,
            )
            mk = oh_pool.tile([P, M], mybir.dt.uint8, tag="mk", name="mk")
            nc.gpsimd.tensor_scalar(
                out=mk[:], in0=dsb[:], scalar1=R, scalar2=None, op0=OP.is_le,
            )
            nc.sync.dma_start(dists_o[t * P:(t + 1) * P, :], dsb[:])
            nc.sync.dma_start(masks_o[t * P:(t + 1) * P, :], mk[:])
            for i in range(2):
                nc.tensor.matmul(
                    sums_ps[i][:], oh[:, i * P:(i + 1) * P], zb[:],
                    start=(t == 0), stop=(t == NT - 1), skip_group_check=True,
                )

        # hard_idx: transpose [P, NT] staging -> [NT, P], cast to int32, store
        ps_idx = tr_pool.tile([P, D], F32, tag="tr", name="ps_idx")
        nc.tensor.transpose(ps_idx[:, 0:P], idx_stage[:], sb_id[:])
        idxT = const.tile([P, P], mybir.dt.int32, tag="idxT")
        nc.vector.tensor_copy(idxT[0:NT, :], ps_idx[0:NT, 0:P])
        nc.sync.dma_start(idx_o.rearrange("(t b) -> t b", b=P), idxT[0:NT, :])
        for i in range(2):
            scp = ep_pool.tile([P, D], F32, tag="sumcp", name="scp")
            nc.scalar.activation(scp[:], sums_ps[i][:], AF.Copy)
            nc.sync.dma_start(sums_o[i * P:(i + 1) * P, :], scp[:])

    nc.compile()
    return nc


def _rne11(x):
    u = np.ascontiguousarray(x, dtype=np.float32).view(np.uint32)
    half = np.uint32(1 << 11)
    lsb = (u >> np.uint32(12)) & np.uint32(1)
    out = (u + half - np.uint32(1) + lsb) & np.uint32(0xFFFFF000)
    return out.view(np.float32)


def make_consts(centers, stats_mean, stats_var):
    import ml_dtypes

    centers = np.asarray(centers, dtype=np.float32)
    mean64 = np.asarray(stats_mean, dtype=np.float64)
    istd64 = 1.0 / np.sqrt(np.asarray(stats_var, dtype=np.float64) + EPS)
    inv_std = (1.0 / np.sqrt(np.asarray(stats_var, np.float32) + np.float32(EPS))).astype(np.float32)
    wbias = (-np.asarray(stats_mean, np.float32) * inv_std).astype(np.float32)

    cT2 = (2.0 * centers.T.astype(np.float64)).astype(np.float32)  # [D, M]
    r_hi = _rne11(cT2)
    r_lo = _rne11((cT2.astype(np.float64) - r_hi).astype(np.float32))
    # corr[m] = sum_d bias_d * Rhi[d, m]; device psum = 2 zw.C - corr
    corr = np.einsum("d,dm->m", wbias.astype(np.float64), r_hi.astype(np.float64))
    cn = np.sum(centers.astype(np.float64) ** 2, axis=1)
    cnadj = (cn - corr).astype(np.float32)

    consts = {
        "rhs_hi": np.ascontiguousarray(r_hi.reshape(NCH, P, M)),
        "rhs_lo": np.ascontiguousarray(r_lo.reshape(NCH, P, M)),
        "cnadj": np.broadcast_to(cnadj, (P, M)).copy(),
        "wscale": np.ascontiguousarray(inv_std.reshape(NCH, P).T),
        "wbias": np.ascontiguousarray(wbias.reshape(NCH, P).T),
        "iota_rep": np.broadcast_to(np.arange(M, dtype=np.float32), (P, M)).copy(),
        "ident": np.eye(P, dtype=np.float32),
    }
    return consts


_CACHE = {}


def _get_prog(BC):
    if BC not in _CACHE:
        _CACHE[BC] = build_program(BC)
    return _CACHE[BC]


def finalize(host_out, centers, stats_mean, stats_var):
    centers = np.asarray(centers, dtype=np.float32)
    hard_idx = host_out["hard_idx"].astype(np.int32)
    counts = np.bincount(hard_idx, minlength=M).astype(np.float64)
    mean64 = np.asarray(stats_mean, dtype=np.float64)
    istd64 = 1.0 / np.sqrt(np.asarray(stats_var, dtype=np.float64) + EPS)
    sums_w = (host_out["sums_raw"] - counts[:, None] * mean64[None, :]) * istd64[None, :]
    mean_i = (sums_w / np.maximum(counts, 1.0)[:, None]).astype(np.float32)
    has = counts > 0
    upd = (np.float32(1.0 - TAU) * centers + np.float32(TAU) * mean_i).astype(np.float32)
    new_centers = np.where(has[:, None], upd, centers)
    return (
        host_out["dists"],
        hard_idx,
        host_out["masks"].astype(bool),
        new_centers,
    )


def kernel(z, centers, stats_mean, stats_var, trace=False):
    import ml_dtypes

    z = np.ascontiguousarray(np.asarray(z, dtype=np.float32))
    BC = z.shape[0] // NCORES
    nc = _get_prog(BC)
    consts = make_consts(centers, stats_mean, stats_var)

    mean32 = np.asarray(stats_mean, np.float32)
    std32 = np.sqrt(np.asarray(stats_var, np.float32) + np.float32(EPS))
    zw = ((z - mean32) / std32).astype(np.float32)
    zn = np.einsum("bd,bd->b", zw.astype(np.float64), zw.astype(np.float64)).astype(np.float32)
    zbf = z.astype(ml_dtypes.bfloat16)

    NT = BC // P
    in_maps = [
        dict(
            consts,
            z=z[i * BC:(i + 1) * BC],
            zbf=zbf[i * BC:(i + 1) * BC],
            zn=np.ascontiguousarray(zn[i * BC:(i + 1) * BC].reshape(NT, P)),
        )
        for i in range(NCORES)
    ]
    res = run_bass_kernel_spmd(nc, in_maps, core_ids=list(range(NCORES)), trace=trace)
    rs = res.results
    host_out = {
        "dists": np.concatenate([r["dists"] for r in rs], axis=0),
        "hard_idx": np.concatenate([r["hard_idx"] for r in rs], axis=0),
        "masks": np.concatenate([r["masks"] for r in rs], axis=0),
        "sums_raw": np.sum(
            np.stack([r["sums"] for r in rs]).astype(np.float64), axis=0
        ),
    }
    out = finalize(host_out, centers, stats_mean, stats_var)
    if trace:
        return out, res
    return out


# revision 8
# speedup vs baseline: 3.5273x; 3.5273x over previous
"""ChartCover (vq_codebook) Trainium2 kernel.

Data-parallel over batch B across 8 NeuronCores; centers/stats replicated.

The TRN2 PE's fast fp32 path (float32r) is TF32-like (11-bit mantissa,
RNE). To keep fp32-class accuracy at ~1 cycle/row the distance matmul is
computed in three f32r/bf16 chains with whitening folded into the center
matrix on the host:
    p = zh @ Uhi + zh @ Ulo + zl @ Ubf - cnk
with U[d,m] = 2*c[m,d]/sigma_d (host-split: Uhi = rne11(U), Ulo =
rne11(U - Uhi), Ubf = bf16(U)), zh = rne11(z^T) (ACT copy rounds),
zl = bf16(z^T - zh) (one DVE subtract), and cnk[m] = |c_m|^2 + 2*k_m
(k_m = mu_w . c_m) delivered into PSUM by a ones-matmul against a
3-row exact residual split of -cnk.
Then d2 = zn - p (zn = row norms of z_w, host-fed), dists = ACT
sqrt(-p + zn), argmin = DVE max8/max_index on p, onehot = DVE
(iota == idx) in bf16, segment sums = onehot^T @ z_bf16 accumulated in
PSUM across all tiles. Masks are derived on the host from the returned
dists (identical comparison to the reference). GPSIMD is unused — its
elementwise throughput (~4us per [128,256] op) made it the critical
path in v1.
"""

from contextlib import ExitStack

import numpy as np

import concourse.bacc as bacc
import concourse.tile as tile
from concourse import mybir
from concourse.bass_utils import run_bass_kernel_spmd

B, D, M = 131072, 512, 256
R = 32.0
TAU = 0.01
EPS = 1e-6
NCORES = 8
P = 128
NCH = D // P  # 4 contraction chunks
F32 = mybir.dt.float32
F32R = mybir.dt.float32r
BF16 = mybir.dt.bfloat16
AF = mybir.ActivationFunctionType
OP = mybir.AluOpType

# ablation flags
CHAIN_CLO = True   # include zh @ Ulo chain (C-side low bits)
CHAIN_ZLO = True   # include zl @ Ubf chain (z-side low bits)
MAX_ON_PSUM = True  # max8/max_index read PSUM directly


def build_program(BC):
    NT = BC // P
    assert NT <= P
    nc = bacc.Bacc("TRN2", target_bir_lowering=False, debug=False)

    z = nc.dram_tensor("z", [BC, D], F32, kind="ExternalInput").ap()
    zbf = nc.dram_tensor("zbf", [BC, D], BF16, kind="ExternalInput").ap()
    zn_in = nc.dram_tensor("zn", [NT, P], F32, kind="ExternalInput").ap()
    u_hi = nc.dram_tensor("u_hi", [NCH, P, M], F32R, kind="ExternalInput").ap()
    u_lo = nc.dram_tensor("u_lo", [NCH, P, M], F32R, kind="ExternalInput").ap()
    u_bf = nc.dram_tensor("u_bf", [NCH, P, M], BF16, kind="ExternalInput").ap()
    cnrows = nc.dram_tensor("cnrows", [P, M], F32R, kind="ExternalInput").ap()
    onesw = nc.dram_tensor("onesw", [P, P], F32R, kind="ExternalInput").ap()
    iota_rep = nc.dram_tensor("iota_rep", [P, M], F32, kind="ExternalInput").ap()
    ident = nc.dram_tensor("ident", [P, P], F32, kind="ExternalInput").ap()

    dists_o = nc.dram_tensor("dists", [BC, M], F32, kind="ExternalOutput").ap()
    idx_o = nc.dram_tensor("hard_idx", [BC], mybir.dt.int32, kind="ExternalOutput").ap()
    sums_o = nc.dram_tensor("sums", [M, D], F32, kind="ExternalOutput").ap()

    with tile.TileContext(nc) as tc, ExitStack() as ctx:
        const = ctx.enter_context(tc.tile_pool(name="const", bufs=1))
        znat_pool = ctx.enter_context(tc.tile_pool(name="znat", bufs=4))
        zbf_pool = ctx.enter_context(tc.tile_pool(name="zbfp", bufs=4))
        zw_pool = ctx.enter_context(tc.tile_pool(name="zw", bufs=3))
        ep_pool = ctx.enter_context(tc.tile_pool(name="ep", bufs=4))
        oh_pool = ctx.enter_context(tc.tile_pool(name="oh", bufs=4))
        small_pool = ctx.enter_context(tc.tile_pool(name="small", bufs=6))
        tr_pool = ctx.enter_context(tc.tile_pool(name="tr", bufs=2, space="PSUM"))
        pd_pool = ctx.enter_context(tc.tile_pool(name="pd", bufs=3, space="PSUM"))
        acc_pool = ctx.enter_context(tc.tile_pool(name="acc", bufs=1, space="PSUM"))

        sb_uhi = const.tile([P, NCH * M], F32R, tag="uhi")
        sb_ulo = const.tile([P, NCH * M], F32R, tag="ulo")
        sb_ubf = const.tile([P, NCH * M], BF16, tag="ubf")
        for c in range(NCH):
            nc.sync.dma_start(sb_uhi[:, c * M:(c + 1) * M], u_hi[c])
            nc.sync.dma_start(sb_ulo[:, c * M:(c + 1) * M], u_lo[c])
            nc.sync.dma_start(sb_ubf[:, c * M:(c + 1) * M], u_bf[c])
        sb_cnrows = const.tile([P, M], F32R, tag="cnrows")
        nc.sync.dma_start(sb_cnrows[:], cnrows[:])
        sb_ones = const.tile([P, P], F32R, tag="ones")
        nc.sync.dma_start(sb_ones[:], onesw[:])
        sb_iota = const.tile([P, M], F32, tag="iota")
        nc.sync.dma_start(sb_iota[:], iota_rep[:])
        sb_id = const.tile([P, P], F32, tag="id")
        nc.sync.dma_start(sb_id[:], ident[:])
        sb_zn = const.tile([P, NT], F32, tag="znc")
        nc.sync.dma_start(sb_zn[:], zn_in.rearrange("t b -> b t"))
        idx_stage = const.tile([P, P], F32, tag="idxstage")

        sums_ps = [acc_pool.tile([P, D], F32, tag=f"sums{i}", name=f"sums_ps{i}")
                   for i in range(2)]

        for t in range(NT):
            zt = znat_pool.tile([P, D], F32, tag="znat", name="zt")
            nc.sync.dma_start(zt[:], z[t * P:(t + 1) * P, :])
            zb = zbf_pool.tile([P, D], BF16, tag="zbf", name="zb")
            nc.sync.dma_start(zb[:], zbf[t * P:(t + 1) * P, :])

            ps_tr = tr_pool.tile([P, D], F32, tag="tr", name="ps_tr")
            for c in range(NCH):
                nc.tensor.transpose(
                    ps_tr[:, c * P:(c + 1) * P], zt[:, c * P:(c + 1) * P], sb_id[:]
                )
            zh = zw_pool.tile([P, D], F32R, tag="zh", name="zh")
            nc.scalar.activation(zh[:], ps_tr[:], AF.Copy)
            if CHAIN_ZLO:
                zl = zw_pool.tile([P, D], BF16, tag="zl", name="zl")
                nc.vector.tensor_tensor(
                    out=zl[:], in0=ps_tr[:], in1=zh[:], op=OP.subtract
                )
            pd = pd_pool.tile([P, M], F32, tag="pd", name="pd")
            nc.tensor.matmul(
                pd[:], sb_ones[:], sb_cnrows[:],
                start=True, stop=False, skip_group_check=True,
            )
            for c in range(NCH):
                sl = slice(c * P, (c + 1) * P)
                ms = slice(c * M, (c + 1) * M)
                last = c == NCH - 1
                nc.tensor.matmul(
                    pd[:], zh[:, sl], sb_uhi[:, ms],
                    start=False, stop=False, skip_group_check=True,
                )
                if CHAIN_CLO:
                    nc.tensor.matmul(
                        pd[:], zh[:, sl], sb_ulo[:, ms],
                        start=False, stop=last and not CHAIN_ZLO,
                        skip_group_check=True,
                    )
                if CHAIN_ZLO:
                    nc.tensor.matmul(
                        pd[:], zl[:, sl], sb_ubf[:, ms],
                        start=False, stop=last, skip_group_check=True,
                    )
            # epilogue
            if MAX_ON_PSUM:
                psrc = pd
            else:
                psrc = ep_pool.tile([P, M], F32, tag="psb", name="psb")
                nc.vector.tensor_scalar(
                    out=psrc[:], in0=pd[:], scalar1=0.0, scalar2=None, op0=OP.add
                )
            maxv = small_pool.tile([P, 8], F32, tag="maxv", name="maxv")
            nc.vector.max(maxv[:], psrc[:])
            idx8 = small_pool.tile([P, 8], mybir.dt.uint32, tag="idx8", name="idx8")
            nc.vector.max_index(idx8[:], maxv[:], psrc[:])
            nc.vector.tensor_copy(idx_stage[:, t:t + 1], idx8[:, 0:1])
            dsb = ep_pool.tile([P, M], F32, tag="dist", name="dsb")
            nc.scalar.activation(
                dsb[:], pd[:], AF.Sqrt, bias=sb_zn[:, t:t + 1], scale=-1.0
            )
            oh = oh_pool.tile([P, M], BF16, tag="oh", name="oh")
            nc.vector.tensor_scalar(
                out=oh[:], in0=sb_iota[:], scalar1=idx_stage[:, t:t + 1],
                scalar2=None, op0=OP.is_equal,
            )
            nc.sync.dma_start(dists_o[t * P:(t + 1) * P, :], dsb[:])
            for i in range(2):
                nc.tensor.matmul(
                    sums_ps[i][:], oh[:, i * P:(i + 1) * P], zb[:],
                    start=(t == 0), stop=(t == NT - 1), skip_group_check=True,
                )

        # hard_idx: transpose [P, NT] staging -> [NT, P], cast to int32, store
        ps_idx = tr_pool.tile([P, D], F32, tag="tr", name="ps_idx")
        nc.tensor.transpose(ps_idx[:, 0:P], idx_stage[:], sb_id[:])
        idxT = const.tile([P, P], mybir.dt.int32, tag="idxT")
        nc.vector.tensor_copy(idxT[0:NT, :], ps_idx[0:NT, 0:P])
        nc.sync.dma_start(idx_o.rearrange("(t b) -> t b", b=P), idxT[0:NT, :])
        for i in range(2):
            scp = ep_pool.tile([P, D], F32, tag="sumcp", name="scp")
            nc.scalar.activation(scp[:], sums_ps[i][:], AF.Copy)
            nc.sync.dma_start(sums_o[i * P:(i + 1) * P, :], scp[:])

    nc.compile()
    return nc


def _rne11(x):
    u = np.ascontiguousarray(x, dtype=np.float32).view(np.uint32)
    half = np.uint32(1 << 11)
    lsb = (u >> np.uint32(12)) & np.uint32(1)
    out = (u + half - np.uint32(1) + lsb) & np.uint32(0xFFFFF000)
    return out.view(np.float32)


def make_consts(centers, stats_mean, stats_var):
    import ml_dtypes

    centers = np.asarray(centers, dtype=np.float32)
    mean64 = np.asarray(stats_mean, dtype=np.float64)
    var64 = np.asarray(stats_var, dtype=np.float64)
    istd64 = 1.0 / np.sqrt(var64 + EPS)

    U = (2.0 * centers.T.astype(np.float64) * istd64[:, None]).astype(np.float32)
    u_hi = _rne11(U)
    u_lo = _rne11((U.astype(np.float64) - u_hi).astype(np.float32))
    u_bf = U.astype(ml_dtypes.bfloat16)

    cn = np.sum(centers.astype(np.float64) ** 2, axis=1)
    # z_w.c = z.(c/sigma) - (mu/sigma).c  => k_m = sum_d (mu_d * istd_d) * c_{m,d}
    k = np.einsum("d,md->m", mean64 * istd64, centers.astype(np.float64))
    cnk = cn + 2.0 * k  # d2 = zn + cnk - 2 z.u
    # 3-row exact residual split of -cnk into f32r rows
    rows = np.zeros((P, M), np.float32)
    resid = (-cnk).copy()
    for r in range(3):
        v = _rne11(resid.astype(np.float32))
        rows[r] = v
        resid = resid - v.astype(np.float64)
    consts = {
        "u_hi": np.ascontiguousarray(u_hi.reshape(NCH, P, M)),
        "u_lo": np.ascontiguousarray(u_lo.reshape(NCH, P, M)),
        "u_bf": np.ascontiguousarray(u_bf.reshape(NCH, P, M)),
        "cnrows": rows,
        "onesw": np.ones((P, P), np.float32),
        "iota_rep": np.broadcast_to(np.arange(M, dtype=np.float32), (P, M)).copy(),
        "ident": np.eye(P, dtype=np.float32),
    }
    return consts


_CACHE = {}


def _get_prog(BC):
    if BC not in _CACHE:
        _CACHE[BC] = build_program(BC)
    return _CACHE[BC]


def finalize(host_out, centers, stats_mean, stats_var):
    centers = np.asarray(centers, dtype=np.float32)
    hard_idx = host_out["hard_idx"].astype(np.int32)
    counts = np.bincount(hard_idx, minlength=M).astype(np.float64)
    mean64 = np.asarray(stats_mean, dtype=np.float64)
    istd64 = 1.0 / np.sqrt(np.asarray(stats_var, dtype=np.float64) + EPS)
    sums_w = (host_out["sums_raw"] - counts[:, None] * mean64[None, :]) * istd64[None, :]
    mean_i = (sums_w / np.maximum(counts, 1.0)[:, None]).astype(np.float32)
    has = counts > 0
    upd = (np.float32(1.0 - TAU) * centers + np.float32(TAU) * mean_i).astype(np.float32)
    new_centers = np.where(has[:, None], upd, centers)
    dists = host_out["dists"]
    masks = dists <= np.float32(R)
    return (dists, hard_idx, masks, new_centers)


def kernel(z, centers, stats_mean, stats_var, trace=False):
    import ml_dtypes

    z = np.ascontiguousarray(np.asarray(z, dtype=np.float32))
    BC = z.shape[0] // NCORES
    nc = _get_prog(BC)
    consts = make_consts(centers, stats_mean, stats_var)

    mean32 = np.asarray(stats_mean, np.float32)
    std32 = np.sqrt(np.asarray(stats_var, np.float32) + np.float32(EPS))
    zw = ((z - mean32) / std32).astype(np.float32)
    zn = np.einsum("bd,bd->b", zw.astype(np.float64), zw.astype(np.float64)).astype(np.float32)
    zbf = z.astype(ml_dtypes.bfloat16)

    NT = BC // P
    in_maps = [
        dict(
            consts,
            z=z[i * BC:(i + 1) * BC],
            zbf=zbf[i * BC:(i + 1) * BC],
            zn=np.ascontiguousarray(zn[i * BC:(i + 1) * BC].reshape(NT, P)),
        )
        for i in range(NCORES)
    ]
    res = run_bass_kernel_spmd(nc, in_maps, core_ids=list(range(NCORES)), trace=trace)
    rs = res.results
    host_out = {
        "dists": np.concatenate([r["dists"] for r in rs], axis=0),
        "hard_idx": np.concatenate([r["hard_idx"] for r in rs], axis=0),
        "sums_raw": np.sum(
            np.stack([r["sums"] for r in rs]).astype(np.float64), axis=0
        ),
    }
    out = finalize(host_out, centers, stats_mean, stats_var)
    if trace:
        return out, res
    return out
